# revision 15
# baseline (speedup 1.0000x reference)
"""Trainium2 Bass kernel for a binary-conv ResNet BasicBlock (training-mode BN).

Reference computation (per nn_BasicBlock_52158082843180):
    out = sign( BN2( conv3x3(sign(BN1(conv3x3(x, sign(w1)))), sign(w2)) ) + x )
with training-mode BatchNorm (batch stats over (N,H,W), biased var, eps=1e-5).

Strategy (8 NeuronCores, data-parallel over batch N=32 -> 4 images/core):
  * conv3x3 as 9 shift-matmuls on TensorE. Both input and output live in a
    58x58 zero-padded per-image layout, so every tap is a constant flat
    offset and rhs windows are contiguous (junk values land only in the two
    pad columns, which are never read back).
  * conv1 input is split as x = fp16(x) + 2^-9 * e4m3((x - fp16(x)) * 2^9).
    The fp16 hi pass is exact vs +-1 weights (e10m11 PE internal); the lo
    residue rides a cheap fp8e4m3 DoubleRow pass (both cin chunks in one
    256-row contraction) with weights +-2^-9, giving ~2^-17 relative input
    accuracy -- ~1.2e3 final sign flips of the ~2.6e3 the 2e-2 rel-err
    budget allows.
  * conv2 uses e4m3 DoubleRow matmuls on the +-1 activations (exact).
    DoubleRow matmuls are LDWEIGHTS-bound (each matmul reloads 256 weight
    columns at the fixed 1.2GHz NX clock), so DR matmuls are emitted at the
    full 406-column psum-chunk width: 2 loads per tap instead of 4.
  * PSUM chunks are [128, 2, 512] bank-pairs; ScalarE evacuates both
    halves in one strided Copy (accum_out = per-channel sum) and DVE
    computes the sum of squares with one fused scalar_tensor_tensor.
  * BatchNorm is sync-BN: 1KB AllReduce of (sum, sumsq), then fused
    BN+sign. Phases run one 128-wide output-channel chunk at a time and the
    BN tail of chunk0 (AllReduce wait, Sqrt, sign passes) is emitted INSIDE
    chunk1's conv pass via image-boundary hooks, so those ScalarE ops never
    sit in front of chunk1's PSUM evacuations in the strict-FIFO ACT queue.
    Stats DMAs ride the ACT hardware DMA queue; bulk streaming (x quarters,
    residual, output) rides the SP queue, so an AllReduce-wait never blocks
    the streaming queue (this starved the PE for 67us in an earlier rev).
  * ~37us of warm-keeper matmuls bridge the BN1(chunk1) AllReduce so the
    PE's HAM clock gate stays at 2.4GHz into conv2.
  * Final pass per image: DVE fused (conv2*scale + residual), ScalarE
    Sign(. + bias), DMA out; overlapped with conv2-chunk1 via the hooks.

kernel(**inputs) takes the full unsharded inputs and returns the full output.
"""

import os
import sys

for _p in ("/root/.axon_site/_ro/trn_rl_repo", "/opt/trn_rl_repo"):
    if os.path.isdir(_p) and _p not in sys.path:
        sys.path.append(_p)

import numpy as np
from contextlib import ExitStack

import concourse.bass as bass
import concourse.bacc as bacc
import concourse.tile as tile
from concourse import mybir, bass_utils

# ---------------------------------------------------------------- constants
N_CORES = 8
B, C, H, W = 32, 256, 56, 56
BSH = B // N_CORES            # images per core
HP, WP = H + 2, W + 2         # padded spatial
FLAT = HP * WP                # 3364 padded pixels per image
NCH = C // 128                # channel chunks of 128 (=2)
NTAP = 3                      # 3x3 kernel
NQ = 4                        # quarters per image (14 output rows each)
RQ = H // NQ                  # output rows per quarter (14)
NCK = 2                       # psum chunks per quarter
RCK = RQ // NCK               # output rows per psum chunk (7)
CKW = RCK * WP                # psum chunk width incl. junk cols (406)
VCK = RCK * W                 # valid elements per chunk (392)
QROWS = RQ + 2                # padded input rows needed per quarter (16)
EPS = 1e-5
LO_SCALE = 512.0              # lo residue stored as e4m3(lo * 2^9)
W_LO = 1.0 / 512.0            # lo-pass weights are +-2^-9 (e4m3 subnormal)

F32 = mybir.dt.float32
FP16 = mybir.dt.float16
BA_DT = mybir.dt.float8e4    # binary activation storage (+-1 exact)
PBW = 512                    # psum bank width (f32), chunk pitch in pairs
QW = QROWS * WP + 2          # staged x-quarter width incl 2 guard elems
N_WARM = 220                 # warm-keeper matmuls bridging the BN1b AllReduce

DR = mybir.MatmulPerfMode.DoubleRow


def _np_dt(dt):
    return np.dtype(mybir.dt.np(dt))


# ---------------------------------------------------------------- program
def build_nc(n_cores=N_CORES):
    nc = bacc.Bacc(
        "TRN2",
        target_bir_lowering=False,
        debug=False,
        enable_asserts=True,
        num_devices=n_cores,
    )
    # per-core DRAM I/O
    xh = nc.dram_tensor("x_hi", [BSH, NCH, 128, FLAT], FP16, kind="ExternalInput").ap()
    xl = nc.dram_tensor("x_l8", [BSH, NCH, 128, FLAT], BA_DT, kind="ExternalInput").ap()
    xr = nc.dram_tensor("x_res", [BSH, NCH, 128, H * W], F32, kind="ExternalInput").ap()
    w1 = nc.dram_tensor("w1t", [NCH, 128, 9, C], FP16, kind="ExternalInput").ap()
    w1l = nc.dram_tensor("w1l8", [128, NCH, 9, C], BA_DT, kind="ExternalInput").ap()
    w2 = nc.dram_tensor("w2t", [128, NCH, 9, C], BA_DT, kind="ExternalInput").ap()
    gb = nc.dram_tensor("gb", [128, 4, NCH], F32, kind="ExternalInput").ap()
    out = nc.dram_tensor("out", [BSH, NCH, 128, H * W], F32, kind="ExternalOutput").ap()

    with tile.TileContext(nc) as tc, ExitStack() as ctx:
        wpool = ctx.enter_context(tc.tile_pool(name="weights", bufs=1))
        big = ctx.enter_context(tc.tile_pool(name="big", bufs=1))
        xqp = ctx.enter_context(tc.tile_pool(name="xq", bufs=1))
        psum = ctx.enter_context(tc.tile_pool(name="psum", bufs=4, space="PSUM"))
        stp = ctx.enter_context(tc.tile_pool(name="stats", bufs=1))
        scrp = ctx.enter_context(tc.tile_pool(name="scr", bufs=1))
        smp = ctx.enter_context(tc.tile_pool(name="small", bufs=1))
        finp = ctx.enter_context(tc.tile_pool(name="fin", bufs=4))
        dram = ctx.enter_context(tc.tile_pool(name="dram", bufs=1, space="DRAM"))

        # ---- persistent tiles
        w1_sb = [wpool.tile([128, 9, C], FP16, tag=f"w1_{c}", name=f"w1_{c}") for c in range(NCH)]
        for c in range(NCH):
            nc.sync.dma_start(out=w1_sb[c][:], in_=w1[c])
        w1l_sb = wpool.tile([128, NCH, 9, C], BA_DT, tag="w1l", name="w1l")
        nc.sync.dma_start(out=w1l_sb[:], in_=w1l[:])
        w2_sb = wpool.tile([128, NCH, 9, C], BA_DT, tag="w2", name="w2")

        # out_sb holds conv1 output (valid pixels only, f32), later reused
        # in-place for conv2 output.
        out_sb = [big.tile([128, BSH, H * W], F32, tag=f"out_{c}", name=f"out_{c}") for c in range(NCH)]
        # binary activations, padded layout, +1 guard element at each end of
        # each cin-chunk plane; merged [128, 2, *] so DoubleRow contracts both
        # chunks in one matmul
        ba_sb = big.tile([128, NCH, BSH * FLAT + 2], BA_DT, tag="ba", name="ba")
        nc.gpsimd.memset(ba_sb[:], 0.0)

        # x-quarter staging: 2 manually-rotated buffer sets. hi fp16 per cin
        # chunk; lo fp8 with both chunks in one tile (DoubleRow rhs layout).
        xq_hi = [[xqp.tile([128, QW], FP16, tag=f"xqh_{b}_{c}", name=f"xqh_{b}_{c}")
                  for c in range(NCH)] for b in range(2)]
        xq_lo = [xqp.tile([128, NCH, QW], BA_DT, tag=f"xql_{b}", name=f"xql_{b}")
                 for b in range(2)]
        for bset in xq_hi:
            for t in bset:
                nc.vector.memset(t[:, 0:1], 0.0)
                nc.vector.memset(t[:, QW - 1:QW], 0.0)
        for t in xq_lo:
            for c in range(NCH):
                nc.vector.memset(t[:, c, 0:1], 0.0)
                nc.vector.memset(t[:, c, QW - 1:QW], 0.0)
        gb_sb = smp.tile([128, 4, NCH], F32, tag="gb", name="gb")
        nc.sync.dma_start(out=gb_sb[:], in_=gb)
        eps_sb = smp.tile([128, 1], F32, tag="eps", name="eps")
        nc.vector.memset(eps_sb[:], EPS)

        NSTAT = BSH * NQ         # one stats slot per (img, quarter)

        def evac(pt, coc, img, q, sums, sqs):
            """Evacuate a [128, 2, 512] psum pair: ScalarE strided Copy (both
            chunks, accum_out = sum) into out_sb, then DVE fused square with
            sum-of-squares accumulator."""
            sidx = img * NQ + q
            dst = out_sb[coc][:, img, q * 2 * VCK:(q + 1) * 2 * VCK]
            dst4 = dst.rearrange("p (c r w) -> p c r w", c=NCK, w=W)
            src4 = pt[:, :, 0:CKW].rearrange(
                "p c (r w) -> p c r w", w=WP)[:, :, :, 1:1 + W]
            nc.scalar.activation(
                out=dst4, in_=src4,
                func=mybir.ActivationFunctionType.Copy,
                accum_out=sums[:, sidx:sidx + 1])
            scr = scrp.tile([128, 2 * VCK], F32, tag="scr", name="scr")
            nc.vector.scalar_tensor_tensor(
                out=scr[:], in0=dst, scalar=1.0, in1=dst,
                op0=mybir.AluOpType.mult, op1=mybir.AluOpType.mult,
                accum_out=sqs[:, sidx:sidx + 1])

        def conv_pass(conv_idx, coc, hooks=None, pre0=False):
            """Emit one conv's matmuls + evacuation + stats for one output-
            channel chunk. conv1 reads streamed x quarters (fp16 hi + fp8e4m3
            DoubleRow lo); conv2 reads ba_sb. hooks[img] is called after that
            image's last quarter (to overlap the other chunk's BN tail)."""
            is1 = conv_idx == 1
            sums = stp.tile([128, NSTAT], F32, tag=f"sum_{coc}",
                            name=f"sum{conv_idx}_{coc}")
            sqs = stp.tile([128, NSTAT], F32, tag=f"sq_{coc}",
                           name=f"sq{conv_idx}_{coc}")
            cosl = slice(coc * 128, (coc + 1) * 128)
            for img in range(BSH):
                for q in range(NQ):
                    if is1:
                        bsel = (img * NQ + q) % 2
                        hi, lo = xq_hi[bsel], xq_lo[bsel]
                        qoff = q * RQ * WP
                        if not (pre0 and img == 0 and q == 0):
                            for cic in range(NCH):
                                nc.sync.dma_start(
                                    out=hi[cic][:, 1:1 + QROWS * WP],
                                    in_=xh[img, cic, :, qoff: qoff + QROWS * WP])
                                nc.sync.dma_start(
                                    out=lo[:, cic, 1:1 + QROWS * WP],
                                    in_=xl[img, cic, :, qoff: qoff + QROWS * WP])
                    pt = psum.tile([128, NCK, PBW], F32, tag="psum", name="pt")
                    started = [False] * NCK
                    if is1:
                        for ky in range(NTAP):
                            for kx in range(NTAP):
                                tap = ky * NTAP + kx
                                # hi fp16 pass, one 128-row chunk per cin half
                                for cic in range(NCH):
                                    lhsT = w1_sb[cic][:, tap, cosl]
                                    for ck in range(NCK):
                                        off = (RCK * ck + ky) * WP + kx
                                        nc.tensor.matmul(
                                            pt[:, ck, 0:CKW], lhsT,
                                            hi[cic][:, off: off + CKW],
                                            start=not started[ck], stop=False)
                                        started[ck] = True
                                # lo fp8 DoubleRow pass, both cin halves, full
                                # 406-wide chunk per matmul (LDWEIGHTS-bound)
                                lhsT8 = w1l_sb[:, :, tap, cosl]
                                for ck in range(NCK):
                                    off = (RCK * ck + ky) * WP + kx
                                    nc.tensor.matmul(
                                        pt[:, ck, 0:CKW], lhsT8,
                                        lo[:, :, off: off + CKW],
                                        perf_mode=DR, start=False,
                                        stop=(tap == 8))
                    else:
                        base = 1 + img * FLAT + q * RQ * WP
                        for ky in range(NTAP):
                            for kx in range(NTAP):
                                tap = ky * NTAP + kx
                                lhsT = w2_sb[:, :, tap, cosl]
                                for ck in range(NCK):
                                    off = base + (RCK * ck + ky) * WP + kx - 1
                                    nc.tensor.matmul(
                                        pt[:, ck, 0:CKW], lhsT,
                                        ba_sb[:, :, off: off + CKW],
                                        perf_mode=DR,
                                        start=(tap == 0), stop=(tap == 8))
                    evac(pt, coc, img, q, sums, sqs)
                if hooks and img in hooks:
                    hooks[img]()
            return sums, sqs

        def bn_push(stats, tag):
            """Reduce stats, DMA to DRAM (ACT hw queue), post the AllReduce."""
            sums, sqs = stats
            pay = smp.tile([128, 2], F32, tag=f"pay{tag}", name=f"pay{tag}")
            nc.vector.reduce_sum(pay[:, 0:1], sums[:], axis=mybir.AxisListType.X)
            nc.vector.reduce_sum(pay[:, 1:2], sqs[:], axis=mybir.AxisListType.X)
            cin = dram.tile([128, 2], F32, tag=f"cin{tag}", name=f"cin{tag}")
            cout_ = dram.tile([128, 2], F32, tag=f"cout{tag}",
                              addr_space="Shared" if n_cores % 2 == 0 else "Local",
                              name=f"ccout{tag}")
            nc.scalar.dma_start(out=cin[:], in_=pay[:])
            nc.gpsimd.collective_compute(
                "AllReduce", mybir.AluOpType.add,
                replica_groups=[list(range(n_cores))],
                ins=[cin.opt()], outs=[cout_.opt()],
            )
            return cout_

        def bn_finish(cout_, tag, bn_idx, coc):
            """Fetch AllReduce result (ACT hw queue) -> scale s_t / bias t_t."""
            ars = smp.tile([128, 2], F32, tag=f"ars{tag}", name=f"ars{tag}")
            nc.scalar.dma_start(out=ars[:], in_=cout_[:])
            gm = smp.tile([128, 1], F32, tag=f"gm{tag}", name=f"gm{tag}")
            gv = smp.tile([128, 1], F32, tag=f"gv{tag}", name=f"gv{tag}")
            s_t = smp.tile([128, 1], F32, tag=f"s{tag}", name=f"s{tag}")
            t_t = smp.tile([128, 1], F32, tag=f"t{tag}", name=f"t{tag}")
            inv = 1.0 / (BSH * n_cores * H * W)
            nc.vector.tensor_scalar_mul(gm[:], ars[:, 0:1], inv)
            nc.vector.tensor_scalar_mul(gv[:], ars[:, 1:2], inv)
            nc.vector.tensor_mul(s_t[:], gm[:], gm[:])          # s_t = gm^2 (scratch)
            nc.vector.tensor_sub(gv[:], gv[:], s_t[:])          # gv = E[x^2]-gm^2
            nc.scalar.activation(out=gv[:], in_=gv[:],
                                 func=mybir.ActivationFunctionType.Sqrt,
                                 bias=eps_sb[:], scale=1.0)      # sqrt(var+eps)
            nc.vector.reciprocal(out=gv[:], in_=gv[:])           # rstd
            gidx, bidx = (0, 1) if bn_idx == 1 else (2, 3)
            gam = gb_sb[:, gidx, coc:coc + 1]
            bet = gb_sb[:, bidx, coc:coc + 1]
            nc.vector.tensor_mul(s_t[:], gv[:], gam)             # s = gamma*rstd
            nc.vector.tensor_mul(t_t[:], gm[:], s_t[:])
            nc.vector.tensor_sub(t_t[:], bet, t_t[:])            # t = beta-gm*s
            return s_t, t_t

        def binact_img(coc, img, s1, t1):
            src = out_sb[coc][:, img, :].rearrange("p (r w) -> p r w", w=W)
            base = 1 + img * FLAT
            # strided [H,W] valid window of the padded image block
            win = ba_sb[:, coc, base + WP: base + WP + H * WP]
            win = win.rearrange("p (r w) -> p r w", w=WP)[:, :, 1:1 + W]
            nc.scalar.activation(out=win, in_=src,
                                 func=mybir.ActivationFunctionType.Sign,
                                 bias=t1[:, 0:1], scale=s1[:, 0:1])

        def final_prefetch(coc):
            """Issue all residual loads up front so they never queue behind
            output stores (SP ring is FIFO; out_i waits on sign_i)."""
            tiles = []
            for img in range(BSH):
                res = finp.tile([128, H * W], F32, tag="xres", name="xres")
                nc.sync.dma_start(out=res[:], in_=xr[img, coc])
                tiles.append(res)
            return tiles

        def final_img(coc, img, s2, t2, res, split=1):
            sl = out_sb[coc][:, img, :]
            hw = H * W // split
            for h in range(split):
                pc = slice(h * hw, (h + 1) * hw)
                # res = conv2*s2 + residual (fused), then sign(. + t2)
                nc.vector.scalar_tensor_tensor(
                    out=res[:, pc], in0=sl[:, pc], scalar=s2[:, 0:1], in1=res[:, pc],
                    op0=mybir.AluOpType.mult, op1=mybir.AluOpType.add)
                nc.scalar.activation(out=res[:, pc], in_=res[:, pc],
                                     func=mybir.ActivationFunctionType.Sign,
                                     bias=t2[:, 0:1], scale=1.0)
                nc.sync.dma_start(out=out[img, coc][:, pc], in_=res[:, pc])

        # ---- conv1: coc0, then coc1 with coc0's BN tail hooked into its
        # image boundaries (so BN1a's AllReduce + binact(coc0) overlap coc1's
        # matmuls without blocking the FIFO ACT queue ahead of coc1's evacs).
        st1a = conv_pass(1, 0)
        ar1a = bn_push(st1a, "1a")
        bn1a = {}

        def fin1a():
            bn1a["st"] = bn_finish(ar1a, "1a", 1, 0)

        def act1a():
            s, t = bn1a["st"]
            for i in range(BSH):
                binact_img(0, i, s, t)

        st1b = conv_pass(1, 1, hooks={1: fin1a, 2: act1a})
        ar1b = bn_push(st1b, "1b")
        s1b, t1b = bn_finish(ar1b, "1b", 1, 1)
        for i in range(BSH):
            binact_img(1, i, s1b, t1b)

        # Warm-keeper matmuls: bridge the BN1b AllReduce + binact window so
        # the PE HAM clock gate never sees an idle MID window (which would
        # halve the clock for the start of conv2). Results are never read.
        dpt = psum.tile([128, NCK, PBW], F32, tag="psum", name="warm")
        for i in range(N_WARM):
            nc.tensor.matmul(dpt[:, 0, 0:CKW], w1_sb[0][:, 0, 0:128],
                             xq_hi[0][0][:, 0:CKW], start=True, stop=True)

        # ---- conv2, same staggering; coc0's BN tail + finals hook into
        # coc1's image boundaries.
        nc.sync.dma_start(out=w2_sb[:], in_=w2[:])
        st2a = conv_pass(2, 0)
        ar2a = bn_push(st2a, "2a")
        bn2a = {}

        def fin2a():
            bn2a["res"] = final_prefetch(0)
            bn2a["st"] = bn_finish(ar2a, "2a", 2, 0)
            s, t = bn2a["st"]
            for i in range(2):
                final_img(0, i, s, t, bn2a["res"][i])

        def act2a():
            s, t = bn2a["st"]
            for i in range(2, BSH):
                final_img(0, i, s, t, bn2a["res"][i])

        st2b = conv_pass(2, 1, hooks={2: fin2a, 3: act2a})
        ar2b = bn_push(st2b, "2b")
        res1 = final_prefetch(1)
        s2b, t2b = bn_finish(ar2b, "2b", 2, 1)
        for i in range(BSH):
            final_img(1, i, s2b, t2b, res1[i], split=2)

    nc.compile()
    return nc


def build_floor_nc():
    """Same I/O signature, near-zero compute: calibrates dispatch overhead."""
    nc = bacc.Bacc("TRN2", target_bir_lowering=False, debug=False,
                   enable_asserts=True, num_devices=N_CORES)
    nc.dram_tensor("x_hi", [BSH, NCH, 128, FLAT], FP16, kind="ExternalInput")
    nc.dram_tensor("x_l8", [BSH, NCH, 128, FLAT], BA_DT, kind="ExternalInput")
    xr = nc.dram_tensor("x_res", [BSH, NCH, 128, H * W], F32,
                        kind="ExternalInput").ap()
    nc.dram_tensor("w1t", [NCH, 128, 9, C], FP16, kind="ExternalInput")
    nc.dram_tensor("w1l8", [128, NCH, 9, C], BA_DT, kind="ExternalInput")
    nc.dram_tensor("w2t", [128, NCH, 9, C], BA_DT, kind="ExternalInput")
    nc.dram_tensor("gb", [128, 4, NCH], F32, kind="ExternalInput")
    out = nc.dram_tensor("out", [BSH, NCH, 128, H * W], F32,
                         kind="ExternalOutput").ap()
    with tile.TileContext(nc) as tc, ExitStack() as ctx:
        p = ctx.enter_context(tc.tile_pool(name="p", bufs=2))
        for img in range(BSH):
            for coc in range(NCH):
                t = p.tile([128, H * W], F32, tag="t", name="t")
                nc.sync.dma_start(out=t[:], in_=xr[img, coc])
                nc.sync.dma_start(out=out[img, coc], in_=t[:])
    nc.compile()
    return nc


# ---------------------------------------------------------------- host side
def preprocess(x, w1, gamma1, beta1, w2, gamma2, beta2):
    """Full inputs -> list of 8 per-core in_maps."""
    x = np.asarray(x, dtype=np.float32)
    xpad = np.zeros((B, C, HP, WP), np.float32)
    xpad[:, :, 1:1 + H, 1:1 + W] = x
    hi = xpad.astype(np.float16)
    lo8 = ((xpad - hi.astype(np.float32)) * LO_SCALE).astype(_np_dt(BA_DT))

    def wprep(w, dt, scale=1.0, merged=False):
        ws = np.sign(np.asarray(w, np.float32)) * scale  # [co, ci, ky, kx]
        wt = np.ascontiguousarray(ws.transpose(1, 2, 3, 0))  # [ci, ky, kx, co]
        wt = wt.reshape(NCH, 128, 9, C)
        if merged:  # [k, j, tap, co] for DoubleRow (contraction row k+128j)
            wt = np.ascontiguousarray(wt.transpose(1, 0, 2, 3))
        return wt.astype(_np_dt(dt))

    w1t = wprep(w1, FP16)
    w1l8 = wprep(w1, BA_DT, scale=W_LO, merged=True)
    w2t = wprep(w2, BA_DT, merged=True)
    gbv = np.stack([np.asarray(a, np.float32) for a in (gamma1, beta1, gamma2, beta2)])
    gb = np.ascontiguousarray(
        gbv.reshape(4, NCH, 128).transpose(2, 0, 1))  # [128, 4, NCH]

    in_maps = []
    for c in range(N_CORES):
        sl = slice(c * BSH, (c + 1) * BSH)
        in_maps.append({
            "x_hi": np.ascontiguousarray(hi[sl]).reshape(BSH, NCH, 128, FLAT),
            "x_l8": np.ascontiguousarray(lo8[sl]).reshape(BSH, NCH, 128, FLAT),
            "x_res": np.ascontiguousarray(x[sl]).reshape(BSH, NCH, 128, H * W),
            "w1t": w1t, "w1l8": w1l8, "w2t": w2t, "gb": gb,
        })
    return in_maps


def postprocess(results):
    outs = [r["out"].reshape(BSH, C, H, W) for r in results]
    return np.concatenate(outs, axis=0).astype(np.float32)


_NC = None


def get_nc():
    global _NC
    if _NC is None:
        _NC = build_nc()
    return _NC


def kernel(**inputs):
    nc = get_nc()
    in_maps = preprocess(**inputs)
    res = bass_utils.run_bass_kernel_spmd(nc, in_maps, core_ids=list(range(N_CORES)))
    return postprocess(res.results)
